# revision 33
# baseline (speedup 1.0000x reference)
"""Trainium2 Bass kernel for 16-head self-attention (D=1024, S=2048, B=2)
with upper-triangular (j >= i) mask and scale 1/head_dim.

Sharding: batch*head-group parallel over 8 cores. Core c handles batch
c//4, heads [4*(c%4), 4*(c%4)+4). Each core computes Q/K/V projections for
its 256 output dims, attention for its 4 heads, and a partial output
projection (its 256 rows of wo). Host sums the 4 partials per batch.

v2: every matmul keeps a K=128 stationary tile ([128, <=128] lhsT).
Alternating K=64 scores with K=128 PV forces a PE pipeline flush per
matmul (row-group mode switch) and caps the column rate at ~1.0/ns vs
2.33/ns uniform (measured). Scores therefore contract over the full
128-partition QT/KT block with the *other* head's Q rows zeroed, which
adds zeros to the products instead of switching modes. Scores/exp/PV all
use exact causal widths (multiples of 128); PV initializes each 512-wide
psum block with its widest (descending-width) diagonal-band matmul so no
zero-fill of e-tails is needed. Softmax normalization: a ones-column in V
yields the denominator row; 1/d comes from an ACT-table Reciprocal on
that psum row, is broadcast across partitions by a selector matmul
(row 64 of the stationary = ones, K=128 so no PE mode switch), and one
DVE multiply scales O'. Junk warmup matmuls ramp the PE p-state during
the initial DMA; weights are DMA'd before x so projections start early.
Output partials are DMA'd as bf16 and summed on the host in f64.
"""

import sys

sys.path.insert(0, "/opt/trn_rl_repo")

import numpy as np

import concourse.bass as bass
import concourse.mybir as mybir
from concourse import tile
from concourse.bass_utils import run_bass_kernel_spmd

# ---------------------------------------------------------------------------
# Workaround: this walrus build supports only 1 sync wait on the SP CTRL
# (drain) instruction; split the TileContext exit drain's waits across
# sequential drains (same-engine program order makes this equivalent).
_MAX_DRAIN_WAITS = 1


def _patched_drain_and_barrier(self, tick_clock, wait_clock):
    from bass_rust import ScopedClock

    nc = self.nc
    drain_inst = nc.sync.drain()
    wait_clock.add_sem_waits(
        drain_inst.ins, ScopedClock({None: tick_clock.global_clock})
    )
    si = drain_inst.ins.sync_info
    if si is not None and len(si.on_wait) > _MAX_DRAIN_WAITS:
        waits = list(si.on_wait)
        si.on_wait = waits[:_MAX_DRAIN_WAITS]
        rest = waits[_MAX_DRAIN_WAITS:]
        while rest:
            chunk, rest = rest[:_MAX_DRAIN_WAITS], rest[_MAX_DRAIN_WAITS:]
            extra = nc.sync.drain()
            esi = extra.ins.sync_info
            if esi is None:
                extra.ins.sync_info = mybir.SyncInfo(on_wait=chunk, on_update=[])
            else:
                esi.on_wait = chunk
    nc.all_engine_barrier()
    assert self.sems is not None
    popped = nc._tile_sem_poison_stack.pop()
    assert popped is self._sem_poison
    nc.clear_and_free_semaphores(list(self.sems.allocated().values()))
    nc.all_engine_barrier()


tile.TileContext._drain_and_barrier = _patched_drain_and_barrier


def _legalize_waits(nc, max_waits=1):
    """This walrus build accepts at most one sync wait per instruction.
    Hoist extra waits onto preceding NoOps on the same engine (same-engine
    program order preserves the gating semantics)."""
    for blk in nc.main_func.blocks:
        out = []
        for inst in blk.instructions:
            si = inst.sync_info
            if si is not None and len(si.on_wait) > max_waits:
                waits = list(si.on_wait)
                si.on_wait = waits[-max_waits:]
                for w in waits[:-max_waits]:
                    nop = mybir.InstNoOp(
                        name=nc.get_next_instruction_name(), ins=[], outs=[]
                    )
                    nop.engine = inst.engine
                    nop.sync_info = mybir.SyncInfo(on_wait=[w], on_update=[])
                    nc.register_instruction(nop)
                    out.append(nop)
            out.append(inst)
        blk.instructions[:] = out


# ---------------------------------------------------------------------------

B, S, D = 2, 2048, 1024
H, HD = 16, 64
SCALE = 1.0 / HD
NCORES = 8
HPC = 4          # heads per core
DHC = HPC * HD   # 256 head-dims per core
P = 128
KC = D // P      # 8 contraction chunks for projections
SC = S // P      # 16 seq chunks of 128
QB = 512         # seq_q block for PV / O-proj
NQB = S // QB    # 4
NWARM = 24       # p-state warmup matmuls

F32 = mybir.dt.float32
BF16 = mybir.dt.bfloat16

_COMPILED = None


def _act_recip(nc, out_ap, in_ap):
    """ACT-engine reciprocal. The bass wrapper refuses Reciprocal for
    accuracy reasons; a softmax denominator only needs ~1e-2 so the table
    version is fine, and it keeps the reciprocal off the busy DVE."""
    sc = nc.scalar
    ins = [
        sc.lower_ap(in_ap),
        mybir.ImmediateValue(dtype=mybir.dt.float32, value=0.0),
        mybir.ImmediateValue(dtype=mybir.dt.float32, value=1.0),
        mybir.ImmediateValue(dtype=mybir.dt.float32, value=0.0),
    ]
    return sc.add_instruction(
        mybir.InstActivation(
            name=nc.get_next_instruction_name(),
            func=mybir.ActivationFunctionType.Reciprocal,
            ins=ins,
            outs=[sc.lower_ap(out_ap)],
        )
    )


def _build_nc():
    nc = bass.Bass("TRN2", target_bir_lowering=False, debug=False,
                   num_devices=NCORES)

    xT = nc.declare_dram_parameter("xT", [D, S], BF16, isOutput=False)
    wq = nc.declare_dram_parameter("wq", [D, DHC], BF16, isOutput=False)
    wk = nc.declare_dram_parameter("wk", [D, DHC], BF16, isOutput=False)
    wv = nc.declare_dram_parameter("wv", [D, DHC], BF16, isOutput=False)
    wo = nc.declare_dram_parameter("wo", [DHC, D], BF16, isOutput=False)
    bq = nc.declare_dram_parameter("bq", [2, P, 1], F32, isOutput=False)
    bk = nc.declare_dram_parameter("bk", [2, P, 1], F32, isOutput=False)
    bv = nc.declare_dram_parameter("bv", [P, DHC], F32, isOutput=False)
    tri = nc.declare_dram_parameter("tri", [P, P], BF16, isOutput=False)
    outT = nc.declare_dram_parameter("outT", [D, S], BF16, isOutput=True)

    with tile.TileContext(nc) as tc:
        dmaq = [nc.sync, nc.scalar]
        dq = [0]

        def dma(out_ap, in_ap):
            eng = dmaq[dq[0] % len(dmaq)]
            dq[0] += 1
            return eng.dma_start(out_ap, in_ap)

        with (
            tc.tile_pool(name="persist", bufs=1) as pp,
            tc.tile_pool(name="stage", bufs=2) as stage,
            tc.tile_pool(name="epool", bufs=6) as epool,
            tc.tile_pool(name="small", bufs=2) as small,
        ):
            # persistent bf16 tensors
            xTb = [pp.tile([P, S], BF16, tag=f"xtb{k}", name=f"xtb{k}") for k in range(KC)]
            wqb = [pp.tile([P, DHC], BF16, tag=f"wqb{k}", name=f"wqb{k}") for k in range(KC)]
            wkb = [pp.tile([P, DHC], BF16, tag=f"wkb{k}", name=f"wkb{k}") for k in range(KC)]
            wvb = [pp.tile([P, DHC], BF16, tag=f"wvb{k}", name=f"wvb{k}") for k in range(KC)]
            wob = [pp.tile([P, D], BF16, tag=f"wob{c}", name=f"wob{c}") for c in range(2)]
            # Q with other-head rows zeroed (keeps scores at K=128)
            QTz = [pp.tile([P, S], BF16, tag=f"qtz{h}", name=f"qtz{h}") for h in range(HPC)]
            KT = [pp.tile([P, S], BF16, tag=f"kt{m}", name=f"kt{m}") for m in range(2)]
            # V with a ones column per head: [h0(64) 1 | h1(64) 1 | ...]
            Vb = [pp.tile([P, HPC * 65], BF16, tag=f"vb{s}", name=f"vb{s}") for s in range(SC)]
            OT = [pp.tile([P, S], BF16, tag=f"ot{m}", name=f"ot{m}") for m in range(2)]
            trib = pp.tile([P, P], BF16, tag="trib")
            sel = pp.tile([P, 64], BF16, tag="sel")
            dsb = pp.tile([P, S], BF16, tag="dsb")
            bq_sb = pp.tile([P, 2], F32, tag="bq")
            bk_sb = pp.tile([P, 2], F32, tag="bk")
            bv_bc = pp.tile([P, DHC], F32, tag="bvbc")
            wlhs = pp.tile([P, P], BF16, tag="wlhs")
            wrhs = pp.tile([P, QB], BF16, tag="wrhs")

            # ---- DMA: weights first so projections can start early ----
            for k in range(KC):
                dma(wqb[k][:], wq[k * P:(k + 1) * P, :])
            for k in range(KC):
                dma(wkb[k][:], wk[k * P:(k + 1) * P, :])
            # x streams column-quarter-major: the nb=0 projection chains
            # need [all k] x [cols 0:512] only, so they start ~4us in
            for q in range(NQB):
                for k in range(KC):
                    dma(xTb[k][:, q * QB:(q + 1) * QB],
                        xT[k * P:(k + 1) * P, q * QB:(q + 1) * QB])
                if q == 0:
                    for k in range(KC):
                        dma(wvb[k][:], wv[k * P:(k + 1) * P, :])
            for c in range(2):
                dma(wob[c][:], wo[c * P:(c + 1) * P, :])
            dma(trib[:], tri[:, :])
            nc.sync.dma_start(bq_sb[:, 0:1], bq[0])
            nc.sync.dma_start(bq_sb[:, 1:2], bq[1])
            nc.sync.dma_start(bk_sb[:, 0:1], bk[0])
            nc.sync.dma_start(bk_sb[:, 1:2], bk[1])
            nc.sync.dma_start(bv_bc[:], bv[:, :])

            # zero the dead half of each QTz and set V ones columns
            for m in range(2):
                nc.gpsimd.memset(QTz[2 * m][64:P, :], 0.0)
                nc.gpsimd.memset(QTz[2 * m + 1][0:64, :], 0.0)
            for s in range(SC):
                ones = Vb[s][:].rearrange("p (h x) -> p h x", h=HPC)[:, :, 64:65]
                nc.gpsimd.memset(ones, 1.0)
            nc.gpsimd.memset(wlhs[:], 0.25)
            nc.gpsimd.memset(wrhs[:], 0.25)
            nc.gpsimd.memset(sel[:], 0.0)
            nc.gpsimd.memset(sel[64:65, :], 1.0)
            nc.gpsimd.memset(dsb[:], 0.0)

            with tc.tile_pool(name="apsum", bufs=6, space="PSUM") as aps:
                # p-state warmup: junk matmuls with no DMA dependency keep
                # the PE busy (and ramping) while the first inputs stream in
                for w in range(NWARM):
                    wps = aps.tile([P, QB], F32, tag="proj", name=f"warm{w}")
                    nc.tensor.matmul(wps[:], wlhs[:], wrhs[:],
                                     start=True, stop=True)

                # projections in seq-quarter order, matching the DMA
                # stream: each nb slot needs only x[:, nb*512:(nb+1)*512]
                for nb in range(NQB):
                    for m in range(2):
                        ps = aps.tile([P, QB], F32, tag="proj")
                        for k in range(KC):
                            nc.tensor.matmul(
                                ps[:],
                                wqb[k][:, m * P:(m + 1) * P],
                                xTb[k][:, nb * QB:(nb + 1) * QB],
                                start=(k == 0),
                                stop=(k == KC - 1),
                            )
                        sl = slice(nb * QB, (nb + 1) * QB)
                        nc.vector.tensor_scalar_add(
                            QTz[2 * m][0:64, sl], ps[0:64, :],
                            bq_sb[0:64, m:m + 1])
                        nc.vector.tensor_scalar_add(
                            QTz[2 * m + 1][64:P, sl], ps[64:P, :],
                            bq_sb[64:P, m:m + 1])
                    for m in range(2):
                        ps = aps.tile([P, QB], F32, tag="proj")
                        for k in range(KC):
                            nc.tensor.matmul(
                                ps[:],
                                wkb[k][:, m * P:(m + 1) * P],
                                xTb[k][:, nb * QB:(nb + 1) * QB],
                                start=(k == 0),
                                stop=(k == KC - 1),
                            )
                        nc.vector.tensor_scalar_add(
                            KT[m][:, nb * QB:(nb + 1) * QB], ps[:],
                            bk_sb[:, m:m + 1])
                    for s in range(4 * nb, 4 * nb + 4):
                        ps = aps.tile([P, DHC], F32, tag="proj")
                        for k in range(KC):
                            nc.tensor.matmul(
                                ps[:],
                                xTb[k][:, s * P:(s + 1) * P],
                                wvb[k][:],
                                start=(k == 0),
                                stop=(k == KC - 1),
                            )
                        vout = Vb[s][:].rearrange("p (h x) -> p h x", h=HPC)[:, :, 0:64]
                        psr = ps[:].rearrange("p (h x) -> p h x", h=HPC)
                        bvr = bv_bc[:].rearrange("p (h x) -> p h x", h=HPC)
                        nc.vector.tensor_add(vout, psr, bvr)

            # ---------------- Phase B: attention per head ----------------
            with (
                tc.tile_pool(name="scpsum", bufs=3, space="PSUM") as scp,
                tc.tile_pool(name="rbpsum", bufs=1, space="PSUM") as rbpp,
                tc.tile_pool(name="opsum", bufs=1, space="PSUM") as opp,
            ):
                for h in range(HPC):
                    m, poff = h // 2, 64 * (h % 2)
                    ops = opp.tile([65, S], F32, tag="oacc", name="oacc")
                    E = {}

                    def pv_emit(j, h=h, ops=ops, E=E):
                        # PV contributions that become available once e[j]
                        # (and for bands, e[j-3..j]) are written
                        vsl = slice(65 * h, 65 * h + 65)
                        if j % 4 == 3:
                            qb = j // 4
                            for j2 in (j, j - 1, j - 2, j - 3):
                                w2 = P * (j2 + 1) - qb * QB
                                sl = slice(qb * QB, qb * QB + w2)
                                nc.tensor.matmul(
                                    ops[:, sl],
                                    Vb[j2][:, vsl],
                                    E[j2][:, sl],
                                    start=(j2 == j),
                                    stop=(j == SC - 1 and j2 == j - 3),
                                    skip_group_check=True,
                                )
                        for qb2 in range(j // 4):
                            sl = slice(qb2 * QB, (qb2 + 1) * QB)
                            nc.tensor.matmul(
                                ops[:, sl],
                                Vb[j][:, vsl],
                                E[j][:, sl],
                                start=False,
                                stop=(j == SC - 1),
                                skip_group_check=True,
                            )

                    for jc in range(SC):
                        W = P * (jc + 1)
                        e = epool.tile([P, S], BF16, tag="e")
                        E[jc] = e
                        # scores S^T[jc] = KT chunk^T . QTz_h (K=128; the
                        # dead rows multiply the zeroed Q half)
                        for c0 in range(0, W, QB):
                            cw = min(QB, W - c0)
                            sc = scp.tile([P, cw], F32, tag="sc")
                            nc.tensor.matmul(
                                sc[:],
                                KT[m][:, jc * P:(jc + 1) * P],
                                QTz[h][:, c0:c0 + cw],
                                start=True,
                                stop=True,
                            )
                            nc.scalar.activation(
                                e[:, c0:c0 + cw],
                                sc[:],
                                mybir.ActivationFunctionType.Exp,
                                scale=SCALE,
                            )
                        # mask the diagonal 128-block post-exp (x0/1)
                        nc.gpsimd.tensor_mul(
                            e[:, W - P:W], e[:, W - P:W], trib[:]
                        )
                        if jc > 0:
                            pv_emit(jc - 1)
                    pv_emit(SC - 1)

                    # normalize with NO ACT involvement: table reloads from
                    # Exp<->Reciprocal switching trigger the HAM k=4/8 duty
                    # throttle (measured), so the exp table must stay
                    # resident. One wide DVE cast evicts O' + denom to sbuf
                    # (releasing the ops psum for the next head), the raw
                    # denominator row is broadcast by the selector matmul,
                    # and the DVE takes the reciprocal from psum directly.
                    if h < HPC - 1:
                        # slow path, fully hidden under the next head's work
                        osb = small.tile([65, S], BF16, tag="osb", bufs=2)
                        nc.vector.tensor_copy(osb[:], ops[:])
                        nc.vector.tensor_copy(dsb[64:65, :], osb[64:65, :])
                        for qb in range(NQB):
                            sl = slice(qb * QB, (qb + 1) * QB)
                            rbp = rbpp.tile([64, QB], F32, tag="rbp")
                            nc.tensor.matmul(
                                rbp[:], sel[:, :], dsb[:, sl],
                                start=True, stop=True)
                            rinv = small.tile([64, QB], F32, tag="rinv", bufs=2)
                            nc.vector.reciprocal(rinv[:], rbp[:])
                            nc.vector.tensor_mul(
                                OT[m][poff:poff + 64, sl], osb[0:64, sl],
                                rinv[:])
                    else:
                        # last head: short ACT-recip chain so phase C isn't
                        # gated on slow DVE reciprocals; its 2 table reloads
                        # land in the DMA-bound output phase
                        for qb in range(NQB):
                            sl = slice(qb * QB, (qb + 1) * QB)
                            _act_recip(nc, dsb[64:65, sl], ops[64:65, sl])
                            rbp = rbpp.tile([64, QB], F32, tag="rbp")
                            nc.tensor.matmul(
                                rbp[:], sel[:, :], dsb[:, sl],
                                start=True, stop=True)
                            rbc = small.tile([64, QB], BF16, tag="rbc", bufs=2)
                            nc.vector.tensor_copy(rbc[:], rbp[:])
                            nc.vector.tensor_mul(
                                OT[m][poff:poff + 64, sl], ops[0:64, sl],
                                rbc[:])

            # ---------------- Phase C: output projection ----------------
            with tc.tile_pool(name="cpsum", bufs=4, space="PSUM") as cps:
                for mo in range(D // P):
                    ot = stage.tile([P, S], BF16, tag="outstage")
                    for qb in range(NQB):
                        ps = cps.tile([P, QB], F32, tag="oproj")
                        for c in range(2):
                            nc.tensor.matmul(
                                ps[:],
                                wob[c][:, mo * P:(mo + 1) * P],
                                OT[c][:, qb * QB:(qb + 1) * QB],
                                start=(c == 0),
                                stop=(c == 1),
                            )
                        if (mo * NQB + qb) % 2 == 0:
                            nc.vector.tensor_copy(
                                ot[:, qb * QB:(qb + 1) * QB], ps[:])
                        else:
                            nc.scalar.copy(ot[:, qb * QB:(qb + 1) * QB], ps[:])
                    dma(outT[mo * P:(mo + 1) * P, :], ot[:])
    _legalize_waits(nc)
    return nc


def _get_nc():
    global _COMPILED
    if _COMPILED is None:
        _COMPILED = _build_nc()
    return _COMPILED


def _make_in_maps(x, wq, bq, wk, bk, wv, bv, wo, bo):
    import ml_dtypes
    bf16 = ml_dtypes.bfloat16
    tri = np.tril(np.ones((P, P), dtype=bf16))
    in_maps = []
    for c in range(NCORES):
        b, g = c // 4, c % 4
        cols = slice(DHC * g, DHC * (g + 1))
        in_maps.append({
            "xT": np.ascontiguousarray(x[b].T).astype(bf16),
            "wq": np.ascontiguousarray(wq[:, cols]).astype(bf16),
            "wk": np.ascontiguousarray(wk[:, cols]).astype(bf16),
            "wv": np.ascontiguousarray(wv[:, cols]).astype(bf16),
            "wo": np.ascontiguousarray(wo[cols, :]).astype(bf16),
            "bq": np.ascontiguousarray(bq[cols]).reshape(2, P, 1),
            "bk": np.ascontiguousarray(bk[cols]).reshape(2, P, 1),
            "bv": np.ascontiguousarray(np.broadcast_to(bv[cols].reshape(1, DHC), (P, DHC))),
            "tri": tri,
        })
    return in_maps


def kernel(x, wq, bq, wk, bk, wv, bv, wo, bo, _trace=False, _trace_kwargs=None):
    x = np.asarray(x, dtype=np.float32)
    assert x.shape == (B, S, D), x.shape
    nc = _get_nc()
    in_maps = _make_in_maps(
        x, np.asarray(wq), np.asarray(bq), np.asarray(wk), np.asarray(bk),
        np.asarray(wv), np.asarray(bv), np.asarray(wo), np.asarray(bo))
    kw = {}
    if _trace:
        kw = dict(trace=True, **(_trace_kwargs or {}))
    res = run_bass_kernel_spmd(nc, in_maps, list(range(NCORES)), **kw)
    out = np.empty((B, S, D), dtype=np.float32)
    for b in range(B):
        acc = np.zeros((D, S), dtype=np.float64)
        for g in range(4):
            acc += res.results[4 * b + g]["outT"].astype(np.float64)
        out[b] = acc.T.astype(np.float32) + np.asarray(bo, dtype=np.float32)
    kernel.last_result = res
    return out


# revision 34
# speedup vs baseline: 1.0058x; 1.0058x over previous
"""Trainium2 Bass kernel for 16-head self-attention (D=1024, S=2048, B=2)
with upper-triangular (j >= i) mask and scale 1/head_dim.

Sharding: batch*head-group parallel over 8 cores. Core c handles batch
c//4, heads [4*(c%4), 4*(c%4)+4). Each core computes Q/K/V projections for
its 256 output dims, attention for its 4 heads, and a partial output
projection (its 256 rows of wo). Host sums the 4 partials per batch.

v2: every matmul keeps a K=128 stationary tile ([128, <=128] lhsT).
Alternating K=64 scores with K=128 PV forces a PE pipeline flush per
matmul (row-group mode switch) and caps the column rate at ~1.0/ns vs
2.33/ns uniform (measured). Scores therefore contract over the full
128-partition QT/KT block with the *other* head's Q rows zeroed, which
adds zeros to the products instead of switching modes. Scores/exp/PV all
use exact causal widths (multiples of 128); PV initializes each 512-wide
psum block with its widest (descending-width) diagonal-band matmul so no
zero-fill of e-tails is needed. Softmax normalization: a ones-column in V
yields the denominator row; 1/d comes from an ACT-table Reciprocal on
that psum row, is broadcast across partitions by a selector matmul
(row 64 of the stationary = ones, K=128 so no PE mode switch), and one
DVE multiply scales O'. Junk warmup matmuls ramp the PE p-state during
the initial DMA; weights are DMA'd before x so projections start early.
Output partials are DMA'd as bf16 and summed on the host in f64.
"""

import sys

sys.path.insert(0, "/opt/trn_rl_repo")

import numpy as np

import concourse.bass as bass
import concourse.mybir as mybir
from concourse import tile
from concourse.bass_utils import run_bass_kernel_spmd

# ---------------------------------------------------------------------------
# Workaround: this walrus build supports only 1 sync wait on the SP CTRL
# (drain) instruction; split the TileContext exit drain's waits across
# sequential drains (same-engine program order makes this equivalent).
_MAX_DRAIN_WAITS = 1


def _patched_drain_and_barrier(self, tick_clock, wait_clock):
    from bass_rust import ScopedClock

    nc = self.nc
    drain_inst = nc.sync.drain()
    wait_clock.add_sem_waits(
        drain_inst.ins, ScopedClock({None: tick_clock.global_clock})
    )
    si = drain_inst.ins.sync_info
    if si is not None and len(si.on_wait) > _MAX_DRAIN_WAITS:
        waits = list(si.on_wait)
        si.on_wait = waits[:_MAX_DRAIN_WAITS]
        rest = waits[_MAX_DRAIN_WAITS:]
        while rest:
            chunk, rest = rest[:_MAX_DRAIN_WAITS], rest[_MAX_DRAIN_WAITS:]
            extra = nc.sync.drain()
            esi = extra.ins.sync_info
            if esi is None:
                extra.ins.sync_info = mybir.SyncInfo(on_wait=chunk, on_update=[])
            else:
                esi.on_wait = chunk
    nc.all_engine_barrier()
    assert self.sems is not None
    popped = nc._tile_sem_poison_stack.pop()
    assert popped is self._sem_poison
    nc.clear_and_free_semaphores(list(self.sems.allocated().values()))
    nc.all_engine_barrier()


tile.TileContext._drain_and_barrier = _patched_drain_and_barrier


def _legalize_waits(nc, max_waits=1):
    """This walrus build accepts at most one sync wait per instruction.
    Hoist extra waits onto preceding NoOps on the same engine (same-engine
    program order preserves the gating semantics)."""
    for blk in nc.main_func.blocks:
        out = []
        for inst in blk.instructions:
            si = inst.sync_info
            if si is not None and len(si.on_wait) > max_waits:
                waits = list(si.on_wait)
                si.on_wait = waits[-max_waits:]
                for w in waits[:-max_waits]:
                    nop = mybir.InstNoOp(
                        name=nc.get_next_instruction_name(), ins=[], outs=[]
                    )
                    nop.engine = inst.engine
                    nop.sync_info = mybir.SyncInfo(on_wait=[w], on_update=[])
                    nc.register_instruction(nop)
                    out.append(nop)
            out.append(inst)
        blk.instructions[:] = out


# ---------------------------------------------------------------------------

B, S, D = 2, 2048, 1024
H, HD = 16, 64
SCALE = 1.0 / HD
NCORES = 8
HPC = 4          # heads per core
DHC = HPC * HD   # 256 head-dims per core
P = 128
KC = D // P      # 8 contraction chunks for projections
SC = S // P      # 16 seq chunks of 128
QB = 512         # seq_q block for PV / O-proj
NQB = S // QB    # 4
NWARM = 24       # p-state warmup matmuls

F32 = mybir.dt.float32
BF16 = mybir.dt.bfloat16

_COMPILED = None


def _act_recip(nc, out_ap, in_ap):
    """ACT-engine reciprocal. The bass wrapper refuses Reciprocal for
    accuracy reasons; a softmax denominator only needs ~1e-2 so the table
    version is fine, and it keeps the reciprocal off the busy DVE."""
    sc = nc.scalar
    ins = [
        sc.lower_ap(in_ap),
        mybir.ImmediateValue(dtype=mybir.dt.float32, value=0.0),
        mybir.ImmediateValue(dtype=mybir.dt.float32, value=1.0),
        mybir.ImmediateValue(dtype=mybir.dt.float32, value=0.0),
    ]
    return sc.add_instruction(
        mybir.InstActivation(
            name=nc.get_next_instruction_name(),
            func=mybir.ActivationFunctionType.Reciprocal,
            ins=ins,
            outs=[sc.lower_ap(out_ap)],
        )
    )


def _build_nc():
    nc = bass.Bass("TRN2", target_bir_lowering=False, debug=False,
                   num_devices=NCORES)

    xT = nc.declare_dram_parameter("xT", [D, S], BF16, isOutput=False)
    wq = nc.declare_dram_parameter("wq", [D, DHC], BF16, isOutput=False)
    wk = nc.declare_dram_parameter("wk", [D, DHC], BF16, isOutput=False)
    wv = nc.declare_dram_parameter("wv", [D, DHC], BF16, isOutput=False)
    wo = nc.declare_dram_parameter("wo", [DHC, D], BF16, isOutput=False)
    bq = nc.declare_dram_parameter("bq", [2, P, 1], F32, isOutput=False)
    bk = nc.declare_dram_parameter("bk", [2, P, 1], F32, isOutput=False)
    bv = nc.declare_dram_parameter("bv", [P, DHC], F32, isOutput=False)
    tri = nc.declare_dram_parameter("tri", [P, P], BF16, isOutput=False)
    outT = nc.declare_dram_parameter("outT", [D, S], BF16, isOutput=True)

    with tile.TileContext(nc) as tc:
        dmaq = [nc.sync, nc.scalar]
        dq = [0]

        def dma(out_ap, in_ap):
            eng = dmaq[dq[0] % len(dmaq)]
            dq[0] += 1
            return eng.dma_start(out_ap, in_ap)

        with (
            tc.tile_pool(name="persist", bufs=1) as pp,
            tc.tile_pool(name="stage", bufs=2) as stage,
            tc.tile_pool(name="epool", bufs=6) as epool,
            tc.tile_pool(name="small", bufs=2) as small,
        ):
            # persistent bf16 tensors
            xTb = [pp.tile([P, S], BF16, tag=f"xtb{k}", name=f"xtb{k}") for k in range(KC)]
            wqb = [pp.tile([P, DHC], BF16, tag=f"wqb{k}", name=f"wqb{k}") for k in range(KC)]
            wkb = [pp.tile([P, DHC], BF16, tag=f"wkb{k}", name=f"wkb{k}") for k in range(KC)]
            wvb = [pp.tile([P, DHC], BF16, tag=f"wvb{k}", name=f"wvb{k}") for k in range(KC)]
            wob = [pp.tile([P, D], BF16, tag=f"wob{c}", name=f"wob{c}") for c in range(2)]
            # Q with other-head rows zeroed (keeps scores at K=128)
            QTz = [pp.tile([P, S], BF16, tag=f"qtz{h}", name=f"qtz{h}") for h in range(HPC)]
            KT = [pp.tile([P, S], BF16, tag=f"kt{m}", name=f"kt{m}") for m in range(2)]
            # V with a ones column per head: [h0(64) 1 | h1(64) 1 | ...]
            Vb = [pp.tile([P, HPC * 65], BF16, tag=f"vb{s}", name=f"vb{s}") for s in range(SC)]
            OT = [pp.tile([P, S], BF16, tag=f"ot{m}", name=f"ot{m}") for m in range(2)]
            trib = pp.tile([P, P], BF16, tag="trib")
            sel = pp.tile([P, 64], BF16, tag="sel")
            dsb = pp.tile([P, S], BF16, tag="dsb")
            bq_sb = pp.tile([P, 2], F32, tag="bq")
            bk_sb = pp.tile([P, 2], F32, tag="bk")
            bv_bc = pp.tile([P, DHC], F32, tag="bvbc")
            wlhs = pp.tile([P, P], BF16, tag="wlhs")
            wrhs = pp.tile([P, QB], BF16, tag="wrhs")

            # ---- DMA: weights first so projections can start early ----
            for k in range(KC):
                dma(wqb[k][:], wq[k * P:(k + 1) * P, :])
            for k in range(KC):
                dma(wkb[k][:], wk[k * P:(k + 1) * P, :])
            # x streams column-quarter-major: the nb=0 projection chains
            # need [all k] x [cols 0:512] only, so they start ~4us in
            for q in range(NQB):
                for k in range(KC):
                    dma(xTb[k][:, q * QB:(q + 1) * QB],
                        xT[k * P:(k + 1) * P, q * QB:(q + 1) * QB])
                if q == 0:
                    for k in range(KC):
                        dma(wvb[k][:], wv[k * P:(k + 1) * P, :])
            for c in range(2):
                dma(wob[c][:], wo[c * P:(c + 1) * P, :])
            dma(trib[:], tri[:, :])
            nc.sync.dma_start(bq_sb[:, 0:1], bq[0])
            nc.sync.dma_start(bq_sb[:, 1:2], bq[1])
            nc.sync.dma_start(bk_sb[:, 0:1], bk[0])
            nc.sync.dma_start(bk_sb[:, 1:2], bk[1])
            nc.sync.dma_start(bv_bc[:], bv[:, :])

            # warmup operands FIRST on the gpsimd queue -- the p-state
            # warm matmuls gate on these, and the big QTz/V memsets would
            # otherwise delay them by >15us
            nc.gpsimd.memset(wlhs[:], 0.25)
            nc.gpsimd.memset(wrhs[:], 0.25)
            # V ones columns (needed by the first V evicts ~12us in)
            for s in range(SC):
                ones = Vb[s][:].rearrange("p (h x) -> p h x", h=HPC)[:, :, 64:65]
                nc.gpsimd.memset(ones, 1.0)
            # zero the dead half of each QTz (first read ~55us in)
            for m in range(2):
                nc.gpsimd.memset(QTz[2 * m][64:P, :], 0.0)
                nc.gpsimd.memset(QTz[2 * m + 1][0:64, :], 0.0)
            nc.gpsimd.memset(sel[:], 0.0)
            nc.gpsimd.memset(sel[64:65, :], 1.0)
            nc.gpsimd.memset(dsb[:], 0.0)

            with tc.tile_pool(name="apsum", bufs=6, space="PSUM") as aps:
                # p-state warmup: junk matmuls with no DMA dependency keep
                # the PE busy (and ramping) while the first inputs stream in
                for w in range(NWARM):
                    wps = aps.tile([P, QB], F32, tag="proj", name=f"warm{w}")
                    nc.tensor.matmul(wps[:], wlhs[:], wrhs[:],
                                     start=True, stop=True)

                # projections in seq-quarter order, matching the DMA
                # stream: each nb slot needs only x[:, nb*512:(nb+1)*512]
                for nb in range(NQB):
                    for m in range(2):
                        ps = aps.tile([P, QB], F32, tag="proj")
                        for k in range(KC):
                            nc.tensor.matmul(
                                ps[:],
                                wqb[k][:, m * P:(m + 1) * P],
                                xTb[k][:, nb * QB:(nb + 1) * QB],
                                start=(k == 0),
                                stop=(k == KC - 1),
                            )
                        sl = slice(nb * QB, (nb + 1) * QB)
                        nc.vector.tensor_scalar_add(
                            QTz[2 * m][0:64, sl], ps[0:64, :],
                            bq_sb[0:64, m:m + 1])
                        nc.vector.tensor_scalar_add(
                            QTz[2 * m + 1][64:P, sl], ps[64:P, :],
                            bq_sb[64:P, m:m + 1])
                    for m in range(2):
                        ps = aps.tile([P, QB], F32, tag="proj")
                        for k in range(KC):
                            nc.tensor.matmul(
                                ps[:],
                                wkb[k][:, m * P:(m + 1) * P],
                                xTb[k][:, nb * QB:(nb + 1) * QB],
                                start=(k == 0),
                                stop=(k == KC - 1),
                            )
                        nc.vector.tensor_scalar_add(
                            KT[m][:, nb * QB:(nb + 1) * QB], ps[:],
                            bk_sb[:, m:m + 1])
                    for s in range(4 * nb, 4 * nb + 4):
                        ps = aps.tile([P, DHC], F32, tag="proj")
                        for k in range(KC):
                            nc.tensor.matmul(
                                ps[:],
                                xTb[k][:, s * P:(s + 1) * P],
                                wvb[k][:],
                                start=(k == 0),
                                stop=(k == KC - 1),
                            )
                        vout = Vb[s][:].rearrange("p (h x) -> p h x", h=HPC)[:, :, 0:64]
                        psr = ps[:].rearrange("p (h x) -> p h x", h=HPC)
                        bvr = bv_bc[:].rearrange("p (h x) -> p h x", h=HPC)
                        nc.vector.tensor_add(vout, psr, bvr)

            # ---------------- Phase B: attention per head ----------------
            with (
                tc.tile_pool(name="scpsum", bufs=3, space="PSUM") as scp,
                tc.tile_pool(name="rbpsum", bufs=1, space="PSUM") as rbpp,
                tc.tile_pool(name="opsum", bufs=1, space="PSUM") as opp,
            ):
                for h in range(HPC):
                    m, poff = h // 2, 64 * (h % 2)
                    ops = opp.tile([65, S], F32, tag="oacc", name="oacc")
                    E = {}

                    def pv_emit(j, h=h, ops=ops, E=E):
                        # PV contributions that become available once e[j]
                        # (and for bands, e[j-3..j]) are written
                        vsl = slice(65 * h, 65 * h + 65)
                        if j % 4 == 3:
                            qb = j // 4
                            for j2 in (j, j - 1, j - 2, j - 3):
                                w2 = P * (j2 + 1) - qb * QB
                                sl = slice(qb * QB, qb * QB + w2)
                                nc.tensor.matmul(
                                    ops[:, sl],
                                    Vb[j2][:, vsl],
                                    E[j2][:, sl],
                                    start=(j2 == j),
                                    stop=(j == SC - 1 and j2 == j - 3),
                                    skip_group_check=True,
                                )
                        for qb2 in range(j // 4):
                            sl = slice(qb2 * QB, (qb2 + 1) * QB)
                            nc.tensor.matmul(
                                ops[:, sl],
                                Vb[j][:, vsl],
                                E[j][:, sl],
                                start=False,
                                stop=(j == SC - 1),
                                skip_group_check=True,
                            )

                    for jc in range(SC):
                        W = P * (jc + 1)
                        e = epool.tile([P, S], BF16, tag="e")
                        E[jc] = e
                        # scores S^T[jc] = KT chunk^T . QTz_h (K=128; the
                        # dead rows multiply the zeroed Q half)
                        for c0 in range(0, W, QB):
                            cw = min(QB, W - c0)
                            sc = scp.tile([P, cw], F32, tag="sc")
                            nc.tensor.matmul(
                                sc[:],
                                KT[m][:, jc * P:(jc + 1) * P],
                                QTz[h][:, c0:c0 + cw],
                                start=True,
                                stop=True,
                            )
                            nc.scalar.activation(
                                e[:, c0:c0 + cw],
                                sc[:],
                                mybir.ActivationFunctionType.Exp,
                                scale=SCALE,
                            )
                        # mask the diagonal 128-block post-exp (x0/1)
                        nc.gpsimd.tensor_mul(
                            e[:, W - P:W], e[:, W - P:W], trib[:]
                        )
                        if jc > 0:
                            pv_emit(jc - 1)
                    pv_emit(SC - 1)

                    # normalize with NO ACT involvement: table reloads from
                    # Exp<->Reciprocal switching trigger the HAM k=4/8 duty
                    # throttle (measured), so the exp table must stay
                    # resident. One wide DVE cast evicts O' + denom to sbuf
                    # (releasing the ops psum for the next head), the raw
                    # denominator row is broadcast by the selector matmul,
                    # and the DVE takes the reciprocal from psum directly.
                    if h < HPC - 1:
                        # slow path, fully hidden under the next head's work
                        osb = small.tile([65, S], BF16, tag="osb", bufs=2)
                        nc.vector.tensor_copy(osb[:], ops[:])
                        nc.vector.tensor_copy(dsb[64:65, :], osb[64:65, :])
                        for qb in range(NQB):
                            sl = slice(qb * QB, (qb + 1) * QB)
                            rbp = rbpp.tile([64, QB], F32, tag="rbp")
                            nc.tensor.matmul(
                                rbp[:], sel[:, :], dsb[:, sl],
                                start=True, stop=True)
                            rinv = small.tile([64, QB], F32, tag="rinv", bufs=2)
                            nc.vector.reciprocal(rinv[:], rbp[:])
                            nc.vector.tensor_mul(
                                OT[m][poff:poff + 64, sl], osb[0:64, sl],
                                rinv[:])
                    else:
                        # last head: short ACT-recip chain so phase C isn't
                        # gated on slow DVE reciprocals; its 2 table reloads
                        # land in the DMA-bound output phase
                        for qb in range(NQB):
                            sl = slice(qb * QB, (qb + 1) * QB)
                            _act_recip(nc, dsb[64:65, sl], ops[64:65, sl])
                            rbp = rbpp.tile([64, QB], F32, tag="rbp")
                            nc.tensor.matmul(
                                rbp[:], sel[:, :], dsb[:, sl],
                                start=True, stop=True)
                            rbc = small.tile([64, QB], BF16, tag="rbc", bufs=2)
                            nc.vector.tensor_copy(rbc[:], rbp[:])
                            nc.vector.tensor_mul(
                                OT[m][poff:poff + 64, sl], ops[0:64, sl],
                                rbc[:])

            # ---------------- Phase C: output projection ----------------
            with tc.tile_pool(name="cpsum", bufs=4, space="PSUM") as cps:
                for mo in range(D // P):
                    ot = stage.tile([P, S], BF16, tag="outstage")
                    for qb in range(NQB):
                        ps = cps.tile([P, QB], F32, tag="oproj")
                        for c in range(2):
                            nc.tensor.matmul(
                                ps[:],
                                wob[c][:, mo * P:(mo + 1) * P],
                                OT[c][:, qb * QB:(qb + 1) * QB],
                                start=(c == 0),
                                stop=(c == 1),
                            )
                        if (mo * NQB + qb) % 2 == 0:
                            nc.vector.tensor_copy(
                                ot[:, qb * QB:(qb + 1) * QB], ps[:])
                        else:
                            nc.scalar.copy(ot[:, qb * QB:(qb + 1) * QB], ps[:])
                    dma(outT[mo * P:(mo + 1) * P, :], ot[:])
    _legalize_waits(nc)
    return nc


def _get_nc():
    global _COMPILED
    if _COMPILED is None:
        _COMPILED = _build_nc()
    return _COMPILED


def _make_in_maps(x, wq, bq, wk, bk, wv, bv, wo, bo):
    import ml_dtypes
    bf16 = ml_dtypes.bfloat16
    tri = np.tril(np.ones((P, P), dtype=bf16))
    in_maps = []
    for c in range(NCORES):
        b, g = c // 4, c % 4
        cols = slice(DHC * g, DHC * (g + 1))
        in_maps.append({
            "xT": np.ascontiguousarray(x[b].T).astype(bf16),
            "wq": np.ascontiguousarray(wq[:, cols]).astype(bf16),
            "wk": np.ascontiguousarray(wk[:, cols]).astype(bf16),
            "wv": np.ascontiguousarray(wv[:, cols]).astype(bf16),
            "wo": np.ascontiguousarray(wo[cols, :]).astype(bf16),
            "bq": np.ascontiguousarray(bq[cols]).reshape(2, P, 1),
            "bk": np.ascontiguousarray(bk[cols]).reshape(2, P, 1),
            "bv": np.ascontiguousarray(np.broadcast_to(bv[cols].reshape(1, DHC), (P, DHC))),
            "tri": tri,
        })
    return in_maps


def kernel(x, wq, bq, wk, bk, wv, bv, wo, bo, _trace=False, _trace_kwargs=None):
    x = np.asarray(x, dtype=np.float32)
    assert x.shape == (B, S, D), x.shape
    nc = _get_nc()
    in_maps = _make_in_maps(
        x, np.asarray(wq), np.asarray(bq), np.asarray(wk), np.asarray(bk),
        np.asarray(wv), np.asarray(bv), np.asarray(wo), np.asarray(bo))
    kw = {}
    if _trace:
        kw = dict(trace=True, **(_trace_kwargs or {}))
    res = run_bass_kernel_spmd(nc, in_maps, list(range(NCORES)), **kw)
    out = np.empty((B, S, D), dtype=np.float32)
    for b in range(B):
        acc = np.zeros((D, S), dtype=np.float64)
        for g in range(4):
            acc += res.results[4 * b + g]["outT"].astype(np.float64)
        out[b] = acc.T.astype(np.float32) + np.asarray(bo, dtype=np.float32)
    kernel.last_result = res
    return out
